# revision 8
# baseline (speedup 1.0000x reference)
"""MoE block (8 experts, top-2) on 8 Trainium2 NeuronCores.

Strategy: expert parallelism. The gate (x @ Wg + bg, 0.01% of total FLOPs)
plus top-2 routing runs on the host as part of the sharding step; each of
the 8 cores then runs one expert's FFN over that expert's tokens:

    yT_e = (relu(X_e @ W1[e] + b1[e]) @ W2[e] + b2[e])^T

Device-side layout keeps activations transposed ([feature, token]) so both
matmuls use natural weight layouts as the stationary operand:

    H^T = W1^T X^T   (contract d=1024,  8 k-tiles)
    Y^T = W2^T H^T   (contract dff=4096, 32 k-tiles)

Operands are bf16 (f32 PSUM accumulation): same 1-col/cycle PE rate as
f32r but LDWEIGHTS gets FWL (hides under the matmul stream) and all HBM
traffic halves. End-to-end rel err ~5e-4, far inside the 2e-2 gate.
X^T and H^T stay resident in SBUF for the whole token capacity while W1
and W2 each stream from HBM exactly once (phase-split). X^T lands as one
DMA per token block so the first matmuls start ~2 us in; W2 streams on
the gpsimd (SWDGE) queue so its prefetch overlaps the tail of the W1
stream. The host applies the top-2 softmax weights and scatters back.
"""

import numpy as np
import ml_dtypes

import concourse.bacc as bacc
import concourse.mybir as mybir
from concourse.tile import TileContext
from concourse.bass_utils import run_bass_kernel_spmd

D = 1024
DFF = 4096
E = 8
TOPK = 2
KD = D // 128      # 8   k-tiles for phase 1
MF = DFF // 128    # 32  dff tiles (phase-1 output / phase-2 contraction)
KF = DFF // 128    # 32
MD = D // 128      # 8   output d tiles

F32 = mybir.dt.float32
BF16 = mybir.dt.bfloat16
BF16NP = ml_dtypes.bfloat16

# hts (bf16) for capacity C needs 64*C bytes/partition; xblk+rings ~50 KiB.
MAX_SINGLEPASS_C = 2048

_KERNEL_CACHE = {}


def _build_singlepass(C, NB, nblk):
    """Per-core program, whole capacity resident: phase 1 (stream W1 once)
    then phase 2 (stream W2 once, prefetched on the SWDGE queue)."""
    assert nblk * NB == C

    nc = bacc.Bacc(None, target_bir_lowering=False)
    xTb = nc.dram_tensor("xTb", [nblk, 128, KD, NB], BF16, kind="ExternalInput")
    w1 = nc.dram_tensor("w1", [MF, 128, KD, 128], BF16, kind="ExternalInput")
    b1c = nc.dram_tensor("b1c", [128, MF], F32, kind="ExternalInput")
    w2 = nc.dram_tensor("w2", [MD, 128, KF, 128], BF16, kind="ExternalInput")
    b2c = nc.dram_tensor("b2c", [128, MD], F32, kind="ExternalInput")
    yT = nc.dram_tensor("yT", [D, C], BF16, kind="ExternalOutput")

    with TileContext(nc) as tc:
        with (
            tc.tile_pool(name="acts", bufs=1) as acts,
            tc.tile_pool(name="wpool", bufs=1) as wpool,
            tc.tile_pool(name="cpool", bufs=1) as cpool,
            tc.tile_pool(name="opool", bufs=1) as opool,
            tc.tile_pool(name="psum", bufs=8, space="PSUM") as psum,
        ):
            # biases via gpsimd (SWDGE) — keeps both HWDGE queues free for
            # the latency-critical streams.
            b1t = cpool.tile([128, MF], F32, name="b1t")
            nc.gpsimd.dma_start(out=b1t[:], in_=b1c[:])
            b2t = cpool.tile([128, MD], F32, name="b2t")
            nc.gpsimd.dma_start(out=b2t[:], in_=b2c[:])

            # X^T: one DMA per token block (all 8 k-tiles, ~0.75 MB) on the
            # scalar HWDGE queue. Block 0 lands first so matmuls start early.
            xblk = []
            for nb in range(nblk):
                t = acts.tile([128, KD, NB], BF16, name=f"xb{nb}", tag=f"xb{nb}")
                nc.scalar.dma_start(out=t[:], in_=xTb[nb])
                xblk.append(t)

            hts = [
                acts.tile([128, C], BF16, name=f"ht{m}", tag=f"ht{m}")
                for m in range(MF)
            ]

            # Defer the W2 prefetch until phase 1 is underway: this gpsimd
            # read of an early H tile serializes (engine FIFO) ahead of the
            # W2 dma_starts below, keeping the startup HBM burst clear for
            # the X blocks phase 1 is waiting on.
            wgate = cpool.tile([128, 1], BF16, name="wgate")
            nc.gpsimd.tensor_copy(wgate[:], hts[0][:, 0:1])

            # phase 1: H^T[m] = relu(sum_k W1[k,m]^T @ X^T[k] + b1[m])
            # W1 tile for m streams as one 256 KB DMA on the sync queue.
            for m in range(MF):
                w1t = wpool.tile([128, KD, 128], BF16, name="w1t", tag="w1t",
                                 bufs=3)
                nc.sync.dma_start(out=w1t[:], in_=w1[m])
                for nb in range(nblk):
                    ns = slice(nb * NB, (nb + 1) * NB)
                    ps = psum.tile([128, 512], F32, name="ps", tag="ps")[:, :NB]
                    for k in range(KD):
                        nc.tensor.matmul(
                            ps, lhsT=w1t[:, k, :], rhs=xblk[nb][:, k, :],
                            start=(k == 0), stop=(k == KD - 1),
                        )
                    nc.scalar.activation(
                        hts[m][:, ns], ps,
                        mybir.ActivationFunctionType.Relu,
                        bias=b1t[:, m:m + 1],
                    )

            # phase 2: Y^T[mo] = sum_k W2[k,mo]^T @ H^T[k] + b2[mo]
            # W2 tile for mo streams as one 1 MB DMA on the gpsimd (SWDGE)
            # queue: the first ring slots prefetch during phase 1 without
            # queueing behind the W1 stream.
            for mo in range(MD):
                w2t = wpool.tile([128, KF, 128], BF16, name="w2t", tag="w2t",
                                 bufs=3)
                nc.gpsimd.dma_start(out=w2t[:], in_=w2[mo])
                for nb in range(nblk):
                    ns = slice(nb * NB, (nb + 1) * NB)
                    ps = psum.tile([128, 512], F32, name="ps2", tag="ps")[:, :NB]
                    for k in range(KF):
                        nc.tensor.matmul(
                            ps, lhsT=w2t[:, k, :], rhs=hts[k][:, ns],
                            start=(k == 0), stop=(k == KF - 1),
                        )
                    ot = opool.tile([128, NB], BF16, name="ot", tag="ot", bufs=4)
                    nc.scalar.activation(
                        ot[:], ps,
                        mybir.ActivationFunctionType.Identity,
                        bias=b2t[:, mo:mo + 1],
                    )
                    nc.scalar.dma_start(
                        out=yT[mo * 128:(mo + 1) * 128, ns], in_=ot[:]
                    )
    nc.compile()
    return nc


def _plan(maxc):
    """Pick capacity/tiling. Blocks must be <= 512 (one PSUM bank of f32)."""
    nblk = max(1, -(-maxc // 512))
    NB = max(256, -(-maxc // nblk))
    C = nblk * NB
    return ("single", C, NB, nblk)


def _get_kernel(plan):
    if plan not in _KERNEL_CACHE:
        kind, C, NB, nblk = plan
        _KERNEL_CACHE[plan] = _build_singlepass(C, NB, nblk)
    return _KERNEL_CACHE[plan]


def kernel(x, Wg, bg, W1, b1, W2, b2):
    x = np.asarray(x, dtype=np.float32)
    Wg = np.asarray(Wg, dtype=np.float32)
    bg = np.asarray(bg, dtype=np.float32)
    W1 = np.asarray(W1, dtype=np.float32)
    b1 = np.asarray(b1, dtype=np.float32)
    W2 = np.asarray(W2, dtype=np.float32)
    b2 = np.asarray(b2, dtype=np.float32)

    fsz = x.shape[:-1]
    xf = x.reshape(-1, D)
    n = xf.shape[0]

    # ---- routing (host): gate -> top-2 -> softmax over the top-2 ----
    gate = xf @ Wg + bg                                   # [N, E] f32
    top2 = np.argsort(-gate, axis=-1, kind="stable")[:, :TOPK]   # desc, ties->low idx
    vals = np.take_along_axis(gate, top2, axis=-1)        # [N, 2] sorted desc
    ex = np.exp(vals - vals[:, :1])
    wts = ex / ex.sum(axis=-1, keepdims=True)             # [N, 2] f32

    idx_lists = []
    wt_lists = []
    counts = np.zeros(E, dtype=np.int64)
    for e in range(E):
        tok, slot = np.nonzero(top2 == e)
        idx_lists.append(tok)
        wt_lists.append(wts[tok, slot])
        counts[e] = tok.shape[0]
    maxc = int(counts.max())

    plan = _plan(maxc)
    _, C, NB, nblk = plan
    assert C <= MAX_SINGLEPASS_C, "capacity beyond single-pass SBUF budget"
    nc = _get_kernel(plan)

    # ---- shard: gather tokens + pre-tile weights per expert ----
    in_maps = []
    for e in range(E):
        xe = np.zeros((C, D), dtype=np.float32)
        xe[:counts[e]] = xf[idx_lists[e]]
        xT = xe.T                                           # [D, C]
        xTb = np.ascontiguousarray(
            xT.reshape(KD, 128, nblk, NB).transpose(2, 1, 0, 3)
        ).astype(BF16NP)                                    # [nblk,128,KD,NB]
        w1h = np.ascontiguousarray(
            W1[e].reshape(KD, 128, MF, 128).transpose(2, 1, 0, 3)
        ).astype(BF16NP)                                    # [MF,128,KD,128]
        w2h = np.ascontiguousarray(
            W2[e].reshape(KF, 128, MD, 128).transpose(2, 1, 0, 3)
        ).astype(BF16NP)                                    # [MD,128,KF,128]
        b1c = np.ascontiguousarray(b1[e].reshape(MF, 128).T)  # [128, MF]
        b2c = np.ascontiguousarray(b2[e].reshape(MD, 128).T)  # [128, MD]
        in_maps.append(
            {"xTb": xTb, "w1": w1h, "b1c": b1c, "w2": w2h, "b2c": b2c}
        )

    res = run_bass_kernel_spmd(nc, in_maps, core_ids=list(range(E)))

    # ---- combine (host): apply top-2 softmax weights, scatter-add ----
    out = np.zeros((n, D), dtype=np.float32)
    for e in range(E):
        ye = res.results[e]["yT"].astype(np.float32).T[:counts[e]]  # [count, D]
        out[idx_lists[e]] += wt_lists[e][:, None] * ye
    return out.reshape(*fsz, D)


# revision 11
# speedup vs baseline: 1.1915x; 1.1915x over previous
"""MoE block (8 experts, top-2) on 8 Trainium2 NeuronCores.

Strategy: expert parallelism. The gate (x @ Wg + bg, 0.01% of total FLOPs)
plus top-2 routing runs on the host as part of the sharding step; each of
the 8 cores then runs one expert's FFN over that expert's tokens:

    yT_e = (relu(X_e @ W1[e] + b1[e]) @ W2[e] + b2[e])^T

Device-side layout keeps activations transposed ([feature, token]) so both
matmuls use natural weight layouts as the stationary operand:

    H^T = W1^T X^T   (contract d=1024,  8 k-tiles)
    Y^T = W2^T H^T   (contract dff=4096, 32 k-tiles)

Operands are bf16 (f32 PSUM accumulation): same 1-col/cycle PE rate as
f32r but LDWEIGHTS gets FWL (hides under the matmul stream) and all HBM
traffic halves. End-to-end rel err ~5e-4, far inside the 2e-2 gate.
X^T and H^T stay resident in SBUF for the whole token capacity while W1
and W2 each stream from HBM exactly once (phase-split). X^T lands as one
DMA per token block so the first matmuls start ~2 us in; W2 streams on
the gpsimd (SWDGE) queue so its prefetch overlaps the tail of the W1
stream. The host applies the top-2 softmax weights and scatters back.
"""

import numpy as np
import ml_dtypes

import concourse.bacc as bacc
import concourse.mybir as mybir
from concourse.tile import TileContext
from concourse.bass_utils import run_bass_kernel_spmd

D = 1024
DFF = 4096
E = 8
TOPK = 2
KD = D // 128      # 8   k-tiles for phase 1
MF = DFF // 128    # 32  dff tiles (phase-1 output / phase-2 contraction)
KF = DFF // 128    # 32
MD = D // 128      # 8   output d tiles

F32 = mybir.dt.float32
BF16 = mybir.dt.bfloat16
BF16NP = ml_dtypes.bfloat16

# Per-partition SBUF: hts 64*C B + xblk 16*C B + resident W2 64 KiB +
# W1 ring/consts ~10 KiB must fit ~208 KiB usable.
MAX_SINGLEPASS_C = 1536

_KERNEL_CACHE = {}


def _build_singlepass(C, NB, nblk):
    """Per-core program, whole capacity resident: phase 1 (stream W1 once)
    then phase 2 (stream W2 once, prefetched on the SWDGE queue)."""
    assert nblk * NB == C

    nc = bacc.Bacc(None, target_bir_lowering=False)
    xTb = nc.dram_tensor("xTb", [nblk, 128, KD, NB], BF16, kind="ExternalInput")
    w1 = nc.dram_tensor("w1", [MF, 128, KD, 128], BF16, kind="ExternalInput")
    b1c = nc.dram_tensor("b1c", [128, MF], F32, kind="ExternalInput")
    w2 = nc.dram_tensor("w2", [MD, 128, KF, 128], BF16, kind="ExternalInput")
    b2c = nc.dram_tensor("b2c", [128, MD], F32, kind="ExternalInput")
    yT = nc.dram_tensor("yT", [D, C], BF16, kind="ExternalOutput")

    with TileContext(nc) as tc:
        with (
            tc.tile_pool(name="acts", bufs=1) as acts,
            tc.tile_pool(name="wpool", bufs=1) as wpool,
            tc.tile_pool(name="cpool", bufs=1) as cpool,
            tc.tile_pool(name="opool", bufs=1) as opool,
            tc.tile_pool(name="psum", bufs=8, space="PSUM") as psum,
        ):
            # biases via gpsimd (SWDGE) — keeps both HWDGE queues free for
            # the latency-critical streams.
            b1t = cpool.tile([128, MF], F32, name="b1t")
            nc.gpsimd.dma_start(out=b1t[:], in_=b1c[:])
            b2t = cpool.tile([128, MD], F32, name="b2t")
            nc.gpsimd.dma_start(out=b2t[:], in_=b2c[:])

            # X^T: one DMA per token block (all 8 k-tiles, ~0.75 MB) on the
            # scalar HWDGE queue. Block 0 lands first so matmuls start early.
            xblk = []
            for nb in range(nblk):
                t = acts.tile([128, KD, NB], BF16, name=f"xb{nb}", tag=f"xb{nb}")
                nc.scalar.dma_start(out=t[:], in_=xTb[nb])
                xblk.append(t)

            # W2 is fully SBUF-resident (8 x 1 MB tiles). Queued on the
            # scalar HWDGE queue BEHIND the X blocks: per-queue FIFO means
            # the X stream gets the whole HBM bandwidth first, then W2
            # streams during phase 1 with no phase-2 stalls at all.
            w2ts = []
            for mo in range(MD):
                t = wpool.tile([128, KF, 128], BF16, name=f"w2_{mo}",
                               tag=f"w2_{mo}")
                nc.scalar.dma_start(out=t[:], in_=w2[mo])
                w2ts.append(t)

            hts = [
                acts.tile([128, C], BF16, name=f"ht{m}", tag=f"ht{m}")
                for m in range(MF)
            ]

            # phase 1: H^T[m] = relu(sum_k W1[k,m]^T @ X^T[k] + b1[m])
            # W1 tile for m streams as one 256 KB DMA on the sync queue.
            for m in range(MF):
                w1t = wpool.tile([128, KD, 128], BF16, name="w1t", tag="w1t",
                                 bufs=3)
                nc.sync.dma_start(out=w1t[:], in_=w1[m])
                for nb in range(nblk):
                    ns = slice(nb * NB, (nb + 1) * NB)
                    ps = psum.tile([128, 512], F32, name="ps", tag="ps")[:, :NB]
                    for k in range(KD):
                        nc.tensor.matmul(
                            ps, lhsT=w1t[:, k, :], rhs=xblk[nb][:, k, :],
                            start=(k == 0), stop=(k == KD - 1),
                        )
                    nc.scalar.activation(
                        hts[m][:, ns], ps,
                        mybir.ActivationFunctionType.Relu,
                        bias=b1t[:, m:m + 1],
                    )

            # phase 2: Y^T[mo] = sum_k W2[k,mo]^T @ H^T[k] + b2[mo]
            for mo in range(MD):
                w2t = w2ts[mo]
                for nb in range(nblk):
                    ns = slice(nb * NB, (nb + 1) * NB)
                    ps = psum.tile([128, 512], F32, name="ps2", tag="ps")[:, :NB]
                    for k in range(KF):
                        nc.tensor.matmul(
                            ps, lhsT=w2t[:, k, :], rhs=hts[k][:, ns],
                            start=(k == 0), stop=(k == KF - 1),
                        )
                    ot = opool.tile([128, NB], BF16, name="ot", tag="ot", bufs=4)
                    nc.scalar.activation(
                        ot[:], ps,
                        mybir.ActivationFunctionType.Identity,
                        bias=b2t[:, mo:mo + 1],
                    )
                    nc.scalar.dma_start(
                        out=yT[mo * 128:(mo + 1) * 128, ns], in_=ot[:]
                    )
    nc.compile()
    return nc


def _plan(maxc):
    """Pick capacity/tiling. Blocks must be <= 512 (one PSUM bank of f32)."""
    nblk = max(1, -(-maxc // 512))
    NB = max(256, -(-maxc // nblk))
    C = nblk * NB
    return ("single", C, NB, nblk)


def _get_kernel(plan):
    if plan not in _KERNEL_CACHE:
        kind, C, NB, nblk = plan
        _KERNEL_CACHE[plan] = _build_singlepass(C, NB, nblk)
    return _KERNEL_CACHE[plan]


def kernel(x, Wg, bg, W1, b1, W2, b2):
    x = np.asarray(x, dtype=np.float32)
    Wg = np.asarray(Wg, dtype=np.float32)
    bg = np.asarray(bg, dtype=np.float32)
    W1 = np.asarray(W1, dtype=np.float32)
    b1 = np.asarray(b1, dtype=np.float32)
    W2 = np.asarray(W2, dtype=np.float32)
    b2 = np.asarray(b2, dtype=np.float32)

    fsz = x.shape[:-1]
    xf = x.reshape(-1, D)
    n = xf.shape[0]

    # ---- routing (host): gate -> top-2 -> softmax over the top-2 ----
    gate = xf @ Wg + bg                                   # [N, E] f32
    top2 = np.argsort(-gate, axis=-1, kind="stable")[:, :TOPK]   # desc, ties->low idx
    vals = np.take_along_axis(gate, top2, axis=-1)        # [N, 2] sorted desc
    ex = np.exp(vals - vals[:, :1])
    wts = ex / ex.sum(axis=-1, keepdims=True)             # [N, 2] f32

    idx_lists = []
    wt_lists = []
    counts = np.zeros(E, dtype=np.int64)
    for e in range(E):
        tok, slot = np.nonzero(top2 == e)
        idx_lists.append(tok)
        wt_lists.append(wts[tok, slot])
        counts[e] = tok.shape[0]
    maxc = int(counts.max())

    plan = _plan(maxc)
    _, C, NB, nblk = plan
    assert C <= MAX_SINGLEPASS_C, "capacity beyond single-pass SBUF budget"
    nc = _get_kernel(plan)

    # ---- shard: gather tokens + pre-tile weights per expert ----
    in_maps = []
    for e in range(E):
        xe = np.zeros((C, D), dtype=np.float32)
        xe[:counts[e]] = xf[idx_lists[e]]
        xT = xe.T                                           # [D, C]
        xTb = np.ascontiguousarray(
            xT.reshape(KD, 128, nblk, NB).transpose(2, 1, 0, 3)
        ).astype(BF16NP)                                    # [nblk,128,KD,NB]
        w1h = np.ascontiguousarray(
            W1[e].reshape(KD, 128, MF, 128).transpose(2, 1, 0, 3)
        ).astype(BF16NP)                                    # [MF,128,KD,128]
        w2h = np.ascontiguousarray(
            W2[e].reshape(KF, 128, MD, 128).transpose(2, 1, 0, 3)
        ).astype(BF16NP)                                    # [MD,128,KF,128]
        b1c = np.ascontiguousarray(b1[e].reshape(MF, 128).T)  # [128, MF]
        b2c = np.ascontiguousarray(b2[e].reshape(MD, 128).T)  # [128, MD]
        in_maps.append(
            {"xTb": xTb, "w1": w1h, "b1c": b1c, "w2": w2h, "b2c": b2c}
        )

    res = run_bass_kernel_spmd(nc, in_maps, core_ids=list(range(E)))

    # ---- combine (host): apply top-2 softmax weights, scatter-add ----
    out = np.zeros((n, D), dtype=np.float32)
    for e in range(E):
        ye = res.results[e]["yT"].astype(np.float32).T[:counts[e]]  # [count, D]
        out[idx_lists[e]] += wt_lists[e][:, None] * ye
    return out.reshape(*fsz, D)
